# revision 1
# baseline (speedup 1.0000x reference)
"""Causal self-attention on 8 Trainium2 NeuronCores (Bass/Tile).

Problem: x[4,2048,1024] @ W_attn[1024,3072] + b_attn -> qkv; 16-head causal
attention; y @ W_proj[1024,1024] + b_proj.

Sharding: 2D over (batch, head-group). Core c = (b = c//2, g = c%2); each
core computes q/k/v for its 8 heads over its batch, flash-style causal
attention (no max subtraction — logits are small — with the softmax
denominator accumulated as a 65th "ones" column of v), then a partial
output projection with its 512-row slice of W_proj. Host adds the two
partials per batch plus b_proj.

Matmul dtypes: qkv + scores run float32r (full-rate fp32); attention-weights
/ v / y / W_proj run bf16 (full-rate, fp32 PSUM accumulation).
"""

import numpy as np

import concourse.bass as bass
import concourse.mybir as mybir
import concourse.tile as tile
from concourse import bacc
from concourse.masks import make_identity
from concourse.bass_utils import run_bass_kernel_spmd

F32 = mybir.dt.float32
F32R = mybir.dt.float32r
BF16 = mybir.dt.bfloat16

B, T, D, H = 4, 2048, 1024, 16
HD = D // H               # 64
N_GROUPS = 2
FQ = D // N_GROUPS        # 512 features (8 heads) per core
N_CORES = B * N_GROUPS

# set by test harness to collect an NTFF trace / HW exec time
TRACE = False
LAST_RESULTS = None


def build_nc(T=T, D=D, FQ=FQ, HD=HD, qk_mm=F32R, av_dt=BF16, pj_dt=BF16, reps=1,
             ps_bufs=4, psy_bufs=2, est_bufs=4, xin_bufs=3, wqk_bufs=3, wv_bufs=3,
             skip=(), merge_exp=False, resident_w=False):
    HLOC = FQ // HD
    P = 128
    DCH = D // P
    NTB = T // P
    TCH = 512
    NTC = T // TCH
    NFB = FQ // P
    QCH = 512
    NQC = T // QCH
    JPQ = QCH // P
    HPB = P // HD
    DOUT_CH = 512
    NDOUT = D // DOUT_CH
    NLC = FQ // P
    scale = 1.0 / float(np.sqrt(HD))

    nc = bacc.Bacc()
    xb = nc.dram_tensor("xb", [T, D], F32, kind="ExternalInput")
    wq = nc.dram_tensor("wq", [D, FQ], qk_mm, kind="ExternalInput")
    wk = nc.dram_tensor("wk", [D, FQ], qk_mm, kind="ExternalInput")
    wv = nc.dram_tensor("wv", [D, FQ], qk_mm, kind="ExternalInput")
    bq = nc.dram_tensor("bq", [FQ], F32, kind="ExternalInput")
    bk = nc.dram_tensor("bk", [FQ], F32, kind="ExternalInput")
    bv = nc.dram_tensor("bv", [FQ], qk_mm, kind="ExternalInput")
    wp = nc.dram_tensor("wp", [FQ, D], F32, kind="ExternalInput")
    out = nc.dram_tensor("out", [T, D], F32, kind="ExternalOutput")

    with tile.TileContext(nc) as tc:
        with (
            tc.tile_pool(name="const", bufs=1) as const,
            tc.tile_pool(name="big", bufs=1) as big,
            tc.tile_pool(name="xin", bufs=xin_bufs) as xin,
            tc.tile_pool(name="xtp", bufs=2) as xtp,
            tc.tile_pool(name="wqk", bufs=wqk_bufs) as wqkp,
            tc.tile_pool(name="wvp", bufs=wv_bufs) as wvp,
            tc.tile_pool(name="est", bufs=est_bufs) as est,
            tc.tile_pool(name="small", bufs=3) as small,
            tc.tile_pool(name="outp", bufs=3) as outp,
            tc.tile_pool(name="ps", bufs=(2 if merge_exp else ps_bufs), space="PSUM") as ps,
            tc.tile_pool(name="psy", bufs=psy_bufs, space="PSUM") as psy,
            tc.tile_pool(name="psc", bufs=1, space="PSUM") as psc,
        ):
            ident = const.tile([P, P], F32)
            make_identity(nc, ident)
            ones_f32 = const.tile([1, P], F32, tag="ones_f32")
            nc.vector.memset(ones_f32, 1.0)
            ones_row = const.tile([1, P], qk_mm)
            nc.vector.tensor_copy(out=ones_row, in_=ones_f32)
            # diagonal-block masks: mask_r[p, f] = 1 if f >= p + P*r else 0
            masks = []
            for r in range(JPQ if not merge_exp else 0):
                m = const.tile([P, QCH], BF16, tag=f"mask{r}")
                nc.gpsimd.memset(m, 1.0)
                nc.gpsimd.affine_select(
                    out=m, in_=m,
                    compare_op=mybir.AluOpType.is_ge,
                    fill=0.0,
                    base=-P * r,
                    pattern=[[1, QCH]],
                    channel_multiplier=-1,
                )
                masks.append(m)
            bq_sb = const.tile([P, NFB], F32, tag="bq")
            nc.sync.dma_start(out=bq_sb, in_=bq.rearrange("(o p) -> p o", p=P))
            bk_sb = const.tile([P, NFB], F32, tag="bk")
            nc.sync.dma_start(out=bk_sb, in_=bk.rearrange("(o p) -> p o", p=P))
            bv_sb = const.tile([1, FQ], qk_mm)
            nc.sync.dma_start(out=bv_sb, in_=bv[None, :])

            wp_sb = big.tile([P, NLC * NDOUT, DOUT_CH], pj_dt, tag="wp")
            for i in range(NLC):
                stage = wvp.tile([P, D], F32, tag="wpstage")
                nc.sync.dma_start(out=stage, in_=wp[i * P:(i + 1) * P, :])
                for o in range(NDOUT):
                    nc.vector.tensor_copy(
                        out=wp_sb[:, i * NDOUT + o, :],
                        in_=stage[:, o * DOUT_CH:(o + 1) * DOUT_CH],
                    )

            if resident_w:
                wq_sb = big.tile([P, DCH, FQ], qk_mm, tag="wq_sb")
                nc.sync.dma_start(
                    out=wq_sb, in_=wq.rearrange("(dc p) f -> p dc f", p=P))
                wk_sb = big.tile([P, DCH, FQ], qk_mm, tag="wk_sb")
                nc.sync.dma_start(
                    out=wk_sb, in_=wk.rearrange("(dc p) f -> p dc f", p=P))
                wv_sb = big.tile([P, DCH, FQ], qk_mm, tag="wv_sb")
                nc.sync.dma_start(out=wv_sb, in_=wv.rearrange("(dc p) f -> p dc f", p=P))

            for _rep in range(reps):
              qT = big.tile([P, NFB, T], qk_mm, tag="qT")       # [f%128, fb, tok]
              kT = big.tile([P, NFB, T], qk_mm, tag="kT")
              v_aug = big.tile([P, NTB, HLOC, HD + 1], av_dt, tag="v")
              yT = big.tile([P, NLC, T], pj_dt, tag="yT")     # [dloc%128, lc, tok]

              nc.vector.memset(v_aug[:, :, :, HD:HD + 1], 1.0)

              # stage A+B: transpose x, project q/k (-> [f, tok]) and v (-> [tok, f])
              for tch in range(NTC):
                  t0 = tch * TCH
                  xT = xtp.tile([P, DCH, TCH], qk_mm, tag="xT")
                  for tb in range(TCH // P):
                      x_tile = xin.tile([P, D], F32, tag="xin")
                      nc.sync.dma_start(
                          out=x_tile, in_=xb[t0 + tb * P: t0 + (tb + 1) * P, :])
                      for d4 in range(DCH // 4):
                          pst = ps.tile([P, 512], F32, tag="ps")
                          for dd in range(4):
                              d = d4 * 4 + dd
                              nc.tensor.transpose(
                                  pst[:, dd * P:(dd + 1) * P],
                                  x_tile[:, d * P:(d + 1) * P], ident)
                          nc.vector.tensor_copy(
                              out=xT[:, d4 * 4:(d4 + 1) * 4, tb * P:(tb + 1) * P],
                              in_=pst.rearrange("p (dd q) -> p dd q", q=P))
                  for (w_dram, bias_sb, dstT, w_res) in (
                          (wq, bq_sb, qT, "q"), (wk, bk_sb, kT, "k")):
                      for fb in range(NFB):
                          if resident_w:
                              wt = (wq_sb if w_res == "q" else wk_sb)[
                                  :, :, fb * P:(fb + 1) * P]
                          else:
                              wt = wqkp.tile([P, DCH, P], qk_mm, tag="wqk")
                              nc.sync.dma_start(
                                  out=wt,
                                  in_=w_dram.rearrange("(dc p) f -> p dc f", p=P)[
                                      :, :, fb * P:(fb + 1) * P],
                              )
                          pq = ps.tile([P, 512], F32, tag="ps")
                          for d in range(DCH):
                              nc.tensor.matmul(
                                  pq[:, :TCH],
                                  wt[:, d, :],
                                  xT[:, d, :],
                                  start=(d == 0), stop=(d == DCH - 1),
                              )
                          nc.vector.tensor_scalar_add(
                              out=dstT[:, fb, t0:t0 + TCH], in0=pq[:, :TCH],
                              scalar1=bias_sb[:, fb:fb + 1],
                          )
                  for tb in range(TCH // P):
                      pv = ps.tile([P, 512], F32, tag="ps")
                      for d in range(DCH):
                          if resident_w:
                              wvt = wv_sb[:, d, :]
                          else:
                              wvt = wvp.tile([P, FQ], qk_mm, tag="wv")
                              nc.sync.dma_start(out=wvt, in_=wv[d * P:(d + 1) * P, :])
                          nc.tensor.matmul(
                              pv[:, :FQ],
                              xT[:, d, tb * P:(tb + 1) * P],
                              wvt,
                              start=(d == 0), stop=False,
                          )
                      nc.tensor.matmul(
                          pv[:, :FQ],
                          ones_row,
                          bv_sb,
                          start=False, stop=True,
                      )
                      tbg = tch * (TCH // P) + tb
                      nc.vector.tensor_copy(
                          out=v_aug[:, tbg, :, 0:HD],
                          in_=pv[:, :FQ].rearrange("p (h d) -> p h d", d=HD),
                      )

              # stage C: causal attention per head; denominator rides as row HD
              if merge_exp:
                mask_cat = const.tile([P, 4 * QCH], BF16, tag="mask_cat")
                for r in range(JPQ):
                    nc.gpsimd.memset(mask_cat[:, r * QCH:(r + 1) * QCH], 1.0)
                    nc.gpsimd.affine_select(
                        out=mask_cat[:, r * QCH:(r + 1) * QCH],
                        in_=mask_cat[:, r * QCH:(r + 1) * QCH],
                        compare_op=mybir.AluOpType.is_ge,
                        fill=0.0,
                        base=-P * r,
                        pattern=[[1, QCH]],
                        channel_multiplier=-1,
                    )
                for h in range(HLOC):
                    fb = h // HPB
                    p0 = (h % HPB) * HD
                    for c in range(NQC):
                        q0 = c * QCH
                        py = psy.tile([P, 512], F32, tag="psy")
                        ngrp = c + 1
                        for g in range(ngrp):
                            pstc = psc.tile([P, 4 * QCH], F32, tag="psc")
                            for jj in range(JPQ):
                                j = g * JPQ + jj
                                nc.tensor.matmul(
                                    pstc[:, jj * QCH:(jj + 1) * QCH],
                                    kT[p0:p0 + HD, fb, j * P:(j + 1) * P],
                                    qT[p0:p0 + HD, fb, q0:q0 + QCH],
                                    start=True, stop=True,
                                )
                            eb = est.tile([P, 4 * QCH], av_dt, tag="est")
                            nc.scalar.activation(
                                out=eb, in_=pstc,
                                func=mybir.ActivationFunctionType.Exp,
                                scale=scale,
                            )
                            if g == ngrp - 1:
                                nc.vector.tensor_mul(out=eb, in0=eb, in1=mask_cat)
                            for jj in range(JPQ):
                                j = g * JPQ + jj
                                nc.tensor.matmul(
                                    py[:HD + 1, :QCH],
                                    v_aug[:, j, h, :],
                                    eb[:, jj * QCH:(jj + 1) * QCH],
                                    start=(j == 0), stop=(j == JPQ * ngrp - 1),
                                )
                        recip = small.tile([1, QCH], F32, tag="recip")
                        nc.vector.reciprocal(out=recip, in_=py[HD:HD + 1, :QCH])
                        bcast = small.tile([HD, QCH], F32, tag="bcast")
                        nc.gpsimd.partition_broadcast(bcast, recip)
                        nc.vector.tensor_mul(
                            out=yT[p0:p0 + HD, fb, q0:q0 + QCH],
                            in0=py[:HD, :QCH],
                            in1=bcast,
                        )
              for h in range(HLOC if not merge_exp else 0):
                  fb = h // HPB
                  p0 = (h % HPB) * HD
                  for c in range(NQC):
                      q0 = c * QCH
                      py = psy.tile([P, 512], F32, tag="psy")
                      nj = JPQ * c + JPQ
                      for j in range(nj):
                          pst = ps.tile([P, 512], F32, tag="ps")
                          nc.tensor.matmul(
                              pst[:, :QCH],
                              kT[p0:p0 + HD, fb, j * P:(j + 1) * P],
                              qT[p0:p0 + HD, fb, q0:q0 + QCH],
                              start=True, stop=True,
                          )
                          e = est.tile([P, QCH], av_dt, tag="est")
                          nc.scalar.activation(
                              out=e, in_=pst[:, :QCH],
                              func=mybir.ActivationFunctionType.Exp,
                              scale=scale,
                          )
                          r = j - JPQ * c
                          if r >= 0:
                              nc.vector.tensor_mul(out=e, in0=e, in1=masks[r])
                          nc.tensor.matmul(
                              py[:HD + 1, :QCH],
                              v_aug[:, j, h, :],
                              e,
                              start=(j == 0), stop=(j == nj - 1),
                          )
                      recip = small.tile([1, QCH], F32, tag="recip")
                      nc.vector.reciprocal(out=recip, in_=py[HD:HD + 1, :QCH])
                      bcast = small.tile([HD, QCH], F32, tag="bcast")
                      nc.gpsimd.partition_broadcast(bcast, recip)
                      nc.vector.tensor_mul(
                          out=yT[p0:p0 + HD, fb, q0:q0 + QCH],
                          in0=py[:HD, :QCH],
                          in1=bcast,
                      )

              # stage D: partial output projection (host adds b_proj)
              for tb in range(NTB):
                  for o in range(NDOUT):
                      po = ps.tile([P, 512], F32, tag="ps")
                      for i in range(NLC):
                          nc.tensor.matmul(
                              po[:, :DOUT_CH],
                              yT[:, i, tb * P:(tb + 1) * P],
                              wp_sb[:, i * NDOUT + o, :],
                              start=(i == 0), stop=(i == NLC - 1),
                          )
                      ot = outp.tile([P, DOUT_CH], F32, tag="out")
                      nc.vector.tensor_copy(out=ot, in_=po[:, :DOUT_CH])
                      nc.sync.dma_start(
                          out=out[tb * P:(tb + 1) * P, o * DOUT_CH:(o + 1) * DOUT_CH],
                          in_=ot,
                      )

    nc.finalize()
    return nc


# default build configuration used by kernel(); _core_inputs casts the
# weight inputs to match QK_DT.
DEFAULT_CFG = dict()
QK_DT = F32R

_NC_CACHE = {}


def _get_nc():
    if "nc" not in _NC_CACHE:
        _NC_CACHE["nc"] = build_nc(**DEFAULT_CFG)
    return _NC_CACHE["nc"]


def _core_inputs(inputs):
    x = np.ascontiguousarray(np.asarray(inputs["x"], dtype=np.float32))
    W = np.asarray(inputs["W_attn"], dtype=np.float32)
    ba = np.asarray(inputs["b_attn"], dtype=np.float32)
    Wp = np.asarray(inputs["W_proj"], dtype=np.float32)
    if QK_DT == BF16:
        import ml_dtypes
        wdt = ml_dtypes.bfloat16
    else:
        wdt = np.float32
    maps = []
    for c in range(N_CORES):
        b, g = c // N_GROUPS, c % N_GROUPS
        s = slice(g * FQ, (g + 1) * FQ)
        maps.append({
            "xb": np.ascontiguousarray(x[b]),
            "wq": np.ascontiguousarray(W[:, 0:D][:, s]).astype(wdt),
            "wk": np.ascontiguousarray(W[:, D:2 * D][:, s]).astype(wdt),
            "wv": np.ascontiguousarray(W[:, 2 * D:3 * D][:, s]).astype(wdt),
            "bq": np.ascontiguousarray(ba[0:D][s]),
            "bk": np.ascontiguousarray(ba[D:2 * D][s]),
            "bv": np.ascontiguousarray(ba[2 * D:3 * D][s]).astype(wdt),
            "wp": np.ascontiguousarray(Wp[s, :]),
        })
    return maps


def kernel(**inputs) -> np.ndarray:
    global LAST_RESULTS
    nc = _get_nc()
    maps = _core_inputs(inputs)
    res = run_bass_kernel_spmd(
        nc, maps, list(range(N_CORES)), trace=TRACE,
        trace_cores=list(range(N_CORES)) if TRACE else None,
    )
    LAST_RESULTS = res
    bp = np.asarray(inputs["b_proj"], dtype=np.float32)
    out = np.empty((B, T, D), dtype=np.float32)
    for b in range(B):
        acc = res.results[b * N_GROUPS]["out"].astype(np.float32).copy()
        for g in range(1, N_GROUPS):
            acc += res.results[b * N_GROUPS + g]["out"]
        out[b] = acc + bp
    return out



# revision 19
# speedup vs baseline: 3.4014x; 3.4014x over previous
"""Causal self-attention on 8 Trainium2 NeuronCores (Bass/Tile).

Problem: x[4,2048,1024] @ W_attn[1024,3072] + b_attn -> qkv; 16-head causal
attention; y @ W_proj[1024,1024] + b_proj.

Sharding: 2D over (batch, head-group). Core c = (b = c//2, g = c%2); each
core computes q/k/v for its 8 heads over its batch, flash-style causal
attention (no max subtraction -- logits are small -- with the softmax
denominator accumulated as a 65th "ones" column of v), then a partial
output projection with its 512-row slice of W_proj. Host adds the two
partials per batch plus b_proj.

v2 kernel (build_nc2):
  - chunk-pipelined emission: per 512-token chunk c, stage B (transpose +
    qkv projection), stage D(c-1) (output projection), stage C (attention
    over key blocks <= c). Per-engine queues are in-order, so interleaving
    emission is what lets ACT's exp run concurrently with PE's projection
    matmuls.
  - resident weights (loaded once into SBUF; no per-chunk reload).
  - head-paired attention: heads (2i, 2i+1) live at partitions 0-63 /
    64-127 of feature block i. Their K=64 QK matmuls are emitted
    adjacently with row tile positions (0,0)/(64,0) so the PE runs them
    concurrently in separate row groups of the array.
  - causal trimming: diagonal key blocks only stream the valid query
    columns (N = 512-128r).
  - paired exp: one ACT instruction covers both heads' scores.
"""

import numpy as np

import concourse.bass as bass
import concourse.mybir as mybir
import concourse.tile as tile
from concourse import bacc
from concourse.masks import make_identity
from concourse.bass_utils import run_bass_kernel_spmd

F32 = mybir.dt.float32
F32R = mybir.dt.float32r
BF16 = mybir.dt.bfloat16

B, T, D, H = 4, 2048, 1024, 16
HD = D // H               # 64
N_GROUPS = 2
FQ = D // N_GROUPS        # 512 features (8 heads) per core
N_CORES = B * N_GROUPS

# set by test harness to collect an NTFF trace / HW exec time
TRACE = False
LAST_RESULTS = None


def build_nc2(reps=1, x_dt=BF16, qk_dt=BF16, av_dt=BF16, pj_dt=BF16,
              ps_bufs=2, py_bufs=1, pj_bufs=2, est_bufs=3, xin_bufs=6,
              out_bufs=3, trim=True, out_dt=BF16):
    """Chunk-pipelined causal attention kernel for one core (8 heads, one
    batch's 2048 tokens, feature half [g*512:(g+1)*512])."""
    P = 128
    TCH = 512                 # token chunk
    NTC = T // TCH            # 4 chunks
    TBC = TCH // P            # 4 token blocks per chunk
    DCH = D // P              # 8 contraction blocks
    NFB = FQ // P             # 4 feature blocks of qT/kT
    HLOC = FQ // HD           # 8 heads on this core
    NPAIR = HLOC // 2         # 4 head pairs
    NTB = T // P              # 16 key blocks total
    NLC = FQ // P             # 4 feature blocks of y
    DOUT_CH = 512
    NDOUT = D // DOUT_CH      # 2
    scale = 1.0 / float(np.sqrt(HD))

    nc = bacc.Bacc()
    xb = nc.dram_tensor("xb", [T, D], x_dt, kind="ExternalInput")
    wq = nc.dram_tensor("wq", [D, FQ], x_dt, kind="ExternalInput")
    wk = nc.dram_tensor("wk", [D, FQ], x_dt, kind="ExternalInput")
    wv = nc.dram_tensor("wv", [D, FQ], x_dt, kind="ExternalInput")
    bq = nc.dram_tensor("bq", [FQ], F32, kind="ExternalInput")
    bk = nc.dram_tensor("bk", [FQ], F32, kind="ExternalInput")
    bv = nc.dram_tensor("bv", [FQ], x_dt, kind="ExternalInput")
    wp = nc.dram_tensor("wp", [FQ, D], pj_dt, kind="ExternalInput")
    out = nc.dram_tensor("out", [T, D], out_dt, kind="ExternalOutput")

    with tile.TileContext(nc) as tc:
        with (
            tc.tile_pool(name="const", bufs=1) as const,
            tc.tile_pool(name="big", bufs=1) as big,
            tc.tile_pool(name="xin", bufs=xin_bufs) as xin,
            tc.tile_pool(name="xtp", bufs=2) as xtp,
            tc.tile_pool(name="est", bufs=est_bufs) as est,
            tc.tile_pool(name="small", bufs=3) as small,
            tc.tile_pool(name="outp", bufs=out_bufs) as outp,
            tc.tile_pool(name="ps", bufs=ps_bufs, space="PSUM") as ps,
            tc.tile_pool(name="pj", bufs=pj_bufs, space="PSUM") as pj,
            tc.tile_pool(name="py", bufs=py_bufs, space="PSUM") as pyp,
        ):
            ident = const.tile([P, P], x_dt)
            make_identity(nc, ident)
            ones_f32 = const.tile([1, P], F32, tag="ones_f32")
            nc.vector.memset(ones_f32, 1.0)
            ones_row = const.tile([1, P], x_dt)
            nc.vector.tensor_copy(out=ones_row, in_=ones_f32)
            # diagonal-block masks: mask_r[p, f] = 1 if f >= p + 128*r else 0
            masks = []
            for r in range(TBC):
                m = const.tile([P, TCH], av_dt, tag=f"mask{r}")
                nc.gpsimd.memset(m, 1.0)
                nc.gpsimd.affine_select(
                    out=m, in_=m,
                    compare_op=mybir.AluOpType.is_ge,
                    fill=0.0,
                    base=-P * r,
                    pattern=[[1, TCH]],
                    channel_multiplier=-1,
                )
                masks.append(m)
            # prefetch chunk 0's x tiles ahead of the weight loads so the
            # transposes (which only need x) start immediately
            xpre = []
            for tb in range(TCH // P):
                xt = xin.tile([P, D], x_dt, tag="xin")
                nc.sync.dma_start(out=xt, in_=xb[tb * P:(tb + 1) * P, :])
                xpre.append(xt)

            # resident weights; wq/wk split by feature half so the first
            # projection matmuls only wait on the first 512KB
            wq_sb = big.tile([P, DCH, FQ], x_dt, tag="wq_sb")
            wk_sb = big.tile([P, DCH, FQ], x_dt, tag="wk_sb")
            for (w_sb, w_dram) in ((wq_sb, wq), (wk_sb, wk)):
                for fh in range(2):
                    f0, f1 = fh * (FQ // 2), (fh + 1) * (FQ // 2)
                    nc.sync.dma_start(
                        out=w_sb[:, :, f0:f1],
                        in_=w_dram.rearrange("(dc p) f -> p dc f", p=P)[:, :, f0:f1])
            wv_sb = big.tile([P, DCH, FQ], x_dt, tag="wv_sb")
            nc.sync.dma_start(out=wv_sb, in_=wv.rearrange("(dc p) f -> p dc f", p=P))
            # biases after the bulk weights: tiny but descriptor-heavy DMAs
            bq_sb = const.tile([P, NFB], F32, tag="bq")
            nc.sync.dma_start(out=bq_sb, in_=bq.rearrange("(o p) -> p o", p=P))
            bk_sb = const.tile([P, NFB], F32, tag="bk")
            nc.sync.dma_start(out=bk_sb, in_=bk.rearrange("(o p) -> p o", p=P))
            bv_sb = const.tile([1, FQ], x_dt)
            nc.sync.dma_start(out=bv_sb, in_=bv[None, :])
            # wp_sb[:, lc, o, :] = wp[lc*128:(lc+1)*128, o*512:(o+1)*512]
            wp_sb = big.tile([P, NLC, NDOUT, DOUT_CH], pj_dt, tag="wp_sb")
            nc.sync.dma_start(
                out=wp_sb,
                in_=wp.rearrange("(lc p) (o q) -> p lc o q", p=P, q=DOUT_CH))

            for _rep in range(reps):
                qT = big.tile([P, NFB, T], qk_dt, tag="qT")     # [f%128, fb, tok]
                kT = big.tile([P, NFB, T], qk_dt, tag="kT")
                v_aug = big.tile([P, NTB, HLOC, HD + 1], av_dt, tag="v")
                yT = big.tile([P, NLC, T], pj_dt, tag="yT")     # [f%128, lc, tok]

                nc.vector.memset(v_aug[:, :, :, HD:HD + 1], 1.0)

                def stage_b(c, xtiles=None):
                    """transpose x chunk, project q/k (-> [f, tok]) and v."""
                    t0 = c * TCH
                    xT = xtp.tile([P, DCH, TCH], x_dt, tag="xT")
                    for tb in range(TBC):
                        if xtiles is not None:
                            x_tile = xtiles[tb]
                        else:
                            x_tile = xin.tile([P, D], x_dt, tag="xin")
                            nc.sync.dma_start(
                                out=x_tile,
                                in_=xb[t0 + tb * P: t0 + (tb + 1) * P, :])
                        for d4 in range(DCH // 4):
                            pst = pj.tile([P, 512], x_dt, tag="pj")
                            for dd in range(4):
                                d = d4 * 4 + dd
                                nc.tensor.transpose(
                                    pst[:, dd * P:(dd + 1) * P],
                                    x_tile[:, d * P:(d + 1) * P], ident)
                            nc.vector.tensor_copy(
                                out=xT[:, d4 * 4:(d4 + 1) * 4, tb * P:(tb + 1) * P],
                                in_=pst.rearrange("p (dd q) -> p dd q", q=P))
                    for (w_sb, bias_sb, dstT) in (
                            (wq_sb, bq_sb, qT), (wk_sb, bk_sb, kT)):
                        for fb in range(NFB):
                            pq = pj.tile([P, 512], F32, tag="pj")
                            for d in range(DCH):
                                nc.tensor.matmul(
                                    pq[:, :TCH],
                                    w_sb[:, d, fb * P:(fb + 1) * P],
                                    xT[:, d, :],
                                    start=(d == 0), stop=(d == DCH - 1),
                                )
                            nc.vector.tensor_scalar_add(
                                out=dstT[:, fb, t0:t0 + TCH], in0=pq[:, :TCH],
                                scalar1=bias_sb[:, fb:fb + 1],
                            )
                    for tb in range(TBC):
                        pv = pj.tile([P, 512], F32, tag="pj")
                        for d in range(DCH):
                            nc.tensor.matmul(
                                pv[:, :FQ],
                                xT[:, d, tb * P:(tb + 1) * P],
                                wv_sb[:, d, :],
                                start=(d == 0), stop=False,
                            )
                        nc.tensor.matmul(
                            pv[:, :FQ], ones_row, bv_sb, start=False, stop=True)
                        nc.vector.tensor_copy(
                            out=v_aug[:, c * TBC + tb, :, 0:HD],
                            in_=pv[:, :FQ].rearrange("p (h d) -> p h d", d=HD),
                        )

                def stage_c_pair(c, hp):
                    """causal attention for query chunk c, head pair hp.

                    Software-pipelined: QK(j+1) is emitted before AV(j) so the
                    in-order PE queue computes the next block's scores while
                    ACT runs exp on the current block."""
                    q0 = c * TCH
                    nj = TBC * c + TBC

                    def q_off_of(j):
                        r = j - TBC * c
                        return 0 if (r < 0 or not trim) else P * r

                    def emit_qk(j):
                        q_off = q_off_of(j)
                        sc = ps.tile([P, 1024], F32, tag="sc")
                        # paired QK: row groups (0,0) and (64,0) run
                        # concurrently on the PE array
                        nc.tensor.matmul(
                            sc[:, q_off:TCH],
                            kT[0:HD, hp, j * P:(j + 1) * P],
                            qT[0:HD, hp, q0 + q_off:q0 + TCH],
                            start=True, stop=True,
                        )
                        nc.tensor.matmul(
                            sc[:, TCH + q_off:TCH + TCH],
                            kT[HD:P, hp, j * P:(j + 1) * P],
                            qT[HD:P, hp, q0 + q_off:q0 + TCH],
                            start=True, stop=True,
                        )
                        e = est.tile([P, 1024], av_dt, tag="est")
                        if q_off == 0:
                            nc.scalar.activation(
                                out=e, in_=sc,
                                func=mybir.ActivationFunctionType.Exp,
                                scale=scale,
                            )
                        else:
                            sc_v = sc.rearrange(
                                "p (t q) -> p t q", t=2)[:, :, q_off:TCH]
                            e_v = e.rearrange(
                                "p (t q) -> p t q", t=2)[:, :, q_off:TCH]
                            nc.scalar.activation(
                                out=e_v, in_=sc_v,
                                func=mybir.ActivationFunctionType.Exp,
                                scale=scale,
                            )
                        r = j - TBC * c
                        if r >= 0:
                            nc.vector.tensor_mul(
                                out=e[:, q_off:TCH], in0=e[:, q_off:TCH],
                                in1=masks[r][:, q_off:TCH])
                            nc.vector.tensor_mul(
                                out=e[:, TCH + q_off:2 * TCH],
                                in0=e[:, TCH + q_off:2 * TCH],
                                in1=masks[r][:, q_off:TCH])
                        return e

                    fb = hp
                    py = pyp.tile([P, 1024], F32, tag="py")
                    e_prev = emit_qk(0)
                    for j in range(nj):
                        e_cur = e_prev
                        if j + 1 < nj:
                            e_prev = emit_qk(j + 1)
                        q_off = q_off_of(j)
                        nc.tensor.matmul(
                            py[:HD + 1, q_off:TCH],
                            v_aug[:, j, 2 * hp, :],
                            e_cur[:, q_off:TCH],
                            start=(j == 0), stop=(j == nj - 1),
                        )
                        nc.tensor.matmul(
                            py[:HD + 1, TCH + q_off:2 * TCH],
                            v_aug[:, j, 2 * hp + 1, :],
                            e_cur[:, TCH + q_off:2 * TCH],
                            start=(j == 0), stop=(j == nj - 1),
                        )
                    if True:
                        recip = small.tile([1, 1024], F32, tag="recip")
                        nc.vector.reciprocal(out=recip, in_=py[HD:HD + 1, :])
                        bcA = small.tile([HD, TCH], F32, tag="bcA")
                        nc.gpsimd.partition_broadcast(bcA, recip[:, 0:TCH])
                        bcB = small.tile([HD, TCH], F32, tag="bcB")
                        nc.gpsimd.partition_broadcast(bcB, recip[:, TCH:2 * TCH])
                        nc.vector.tensor_mul(
                            out=yT[0:HD, fb, q0:q0 + TCH],
                            in0=py[:HD, 0:TCH], in1=bcA)
                        nc.vector.tensor_mul(
                            out=yT[HD:P, fb, q0:q0 + TCH],
                            in0=py[:HD, TCH:2 * TCH], in1=bcB)

                def d_unit(tbg, o):
                    """one output-projection tile: tokens [tbg*128, +128),
                    output features [o*512, +512)."""
                    po = pj.tile([P, 512], F32, tag="pj")
                    for lc in range(NLC):
                        nc.tensor.matmul(
                            po[:, :DOUT_CH],
                            yT[:, lc, tbg * P:(tbg + 1) * P],
                            wp_sb[:, lc, o, :],
                            start=(lc == 0), stop=(lc == NLC - 1),
                        )
                    ot = outp.tile([P, DOUT_CH], out_dt, tag="out")
                    nc.vector.tensor_copy(out=ot, in_=po[:, :DOUT_CH])
                    nc.sync.dma_start(
                        out=out[tbg * P:(tbg + 1) * P,
                                o * DOUT_CH:(o + 1) * DOUT_CH],
                        in_=ot,
                    )

                # deferred output-projection queue: D units become ready once
                # their chunk's attention is done; they are drip-fed between
                # attention pairs as PE filler while ACT works through exp
                ready_d = []

                def emit_d(n):
                    for _ in range(min(n, len(ready_d))):
                        d_unit(*ready_d.pop(0))

                for c in range(NTC):
                    stage_b(c, xtiles=xpre if (c == 0 and _rep == 0) else None)
                    emit_d(2)
                    for hp in range(NPAIR):
                        stage_c_pair(c, hp)
                        emit_d(1 if c < NTC - 1 else 3)
                    ready_d += [(c * TBC + tb, o)
                                for tb in range(TBC) for o in range(NDOUT)]
                emit_d(len(ready_d))

    nc.finalize()
    return nc


DEFAULT_CFG = dict()

_NC_CACHE = {}


def _get_nc():
    if "nc" not in _NC_CACHE:
        _NC_CACHE["nc"] = build_nc2(**DEFAULT_CFG)
    return _NC_CACHE["nc"]


def _core_inputs(inputs, x_bf16=True, pj_bf16=True):
    import ml_dtypes
    bf = ml_dtypes.bfloat16
    xdt = bf if x_bf16 else np.float32
    pdt = bf if pj_bf16 else np.float32
    x = np.ascontiguousarray(np.asarray(inputs["x"], dtype=np.float32))
    W = np.asarray(inputs["W_attn"], dtype=np.float32)
    ba = np.asarray(inputs["b_attn"], dtype=np.float32)
    Wp = np.asarray(inputs["W_proj"], dtype=np.float32)
    maps = []
    for c in range(N_CORES):
        b, g = c // N_GROUPS, c % N_GROUPS
        s = slice(g * FQ, (g + 1) * FQ)
        maps.append({
            "xb": np.ascontiguousarray(x[b]).astype(xdt),
            "wq": np.ascontiguousarray(W[:, 0:D][:, s]).astype(xdt),
            "wk": np.ascontiguousarray(W[:, D:2 * D][:, s]).astype(xdt),
            "wv": np.ascontiguousarray(W[:, 2 * D:3 * D][:, s]).astype(xdt),
            "bq": np.ascontiguousarray(ba[0:D][s]),
            "bk": np.ascontiguousarray(ba[D:2 * D][s]),
            "bv": np.ascontiguousarray(ba[2 * D:3 * D][s]).astype(xdt),
            "wp": np.ascontiguousarray(Wp[s, :]).astype(pdt),
        })
    return maps


def kernel(**inputs) -> np.ndarray:
    global LAST_RESULTS
    nc = _get_nc()
    maps = _core_inputs(inputs)
    res = run_bass_kernel_spmd(
        nc, maps, list(range(N_CORES)), trace=TRACE,
        trace_cores=list(range(N_CORES)) if TRACE else None,
    )
    LAST_RESULTS = res
    bp = np.asarray(inputs["b_proj"], dtype=np.float32)
    out = np.empty((B, T, D), dtype=np.float32)
    for b in range(B):
        acc = res.results[b * N_GROUPS]["out"].astype(np.float32).copy()
        for g in range(1, N_GROUPS):
            acc += res.results[b * N_GROUPS + g]["out"]
        out[b] = acc + bp
    return out
